# revision 14
# baseline (speedup 1.0000x reference)
"""Trainium2 Bass kernel for: out = relu(einsum('bcs,cs->bs', x, w) + bias).

Full shapes: x [32, 2048, 4096] f32, w [2048, 4096] f32, bias [4096] f32.
Sharding: the s-axis (4096) is split across 8 cores (512 each); each core
produces out[:, s_slice], gather = concat.

The kernel is HBM-bound (per-core DMA caps at ~400 GB/s regardless of
queue count), so precision is spent where it buys bandwidth: channel
blocks 0..9 ship as bf16, blocks 10..15 as fp8 e3m4 (1-3-4, bias 3,
exp=7 reserved -> clamp at 15.5). Measured end-to-end rel l2 error
~8.6e-3 against the 2e-2 budget. Per-core traffic: 54 MiB.

Engine split per batch (partitions = channel-within-block, free = cb*s):
  sync DMA  fp8 tiles grouped 4 batches per transfer (12 KiB lines),
            bf16 tile per batch (10 KiB lines)
  ACT       upconvert fp8 [128, 3072] -> bf16 staging (fp8 runs 1x on
            DVE, so the idle ACT engine eats the cast instead)
  DVE       xbf *= w[0:10]; up *= w[10:16]   (bf16 2x mode; the Pool
            engine measured 4.6 ns/elem on tensor ops plus heavy SBUF
            interference, so it gets no multiply work)
  PE        bf16 ones-matmul per c-block accumulating the 128-partition
            reduction into PSUM [1,512]; bf16 K=1 bias matmul opens the
            group
  ACT       relu PSUM -> out_sb row b (deferred 3 batches so its
            wait-on-PE cannot stall the upconvert chain)
  scalar DMA out rows 0..23 drained mid-stream, 24..31 at the end
"""

import numpy as np
import ml_dtypes

B, C, S_FULL = 32, 2048, 4096
N_CORES = 8
S = S_FULL // N_CORES          # 512 s-values per core
P = 128                        # SBUF partitions
CB = C // P                    # 16 channel blocks
NBF = 10                       # blocks 0..9 bf16
NF8 = CB - NBF                 # blocks 10..15 fp8 e3m4
G = 4                          # batches per fp8 DMA group
RELU_LAG = 3
EARLY = 24                     # rows drained mid-stream

_nc_cache = {}


def _build():
    import concourse.bacc as bacc
    import concourse.mybir as mybir
    import concourse.tile as tile

    f32 = mybir.dt.float32
    bf16 = mybir.dt.bfloat16
    e3 = mybir.dt.float8e3
    u8 = mybir.dt.uint8
    nc = bacc.Bacc(
        "TRN2",
        target_bir_lowering=False,
        debug=False,
        enable_asserts=False,
        num_devices=N_CORES,
    )

    # Host pre-packs everything into SBUF layout. fp8 is stored grouped:
    # [B/G, P, G*NF8*S] so one DMA covers G batches with 12 KiB lines.
    xbf = nc.dram_tensor("xbf", [B, P, NBF * S], bf16, kind="ExternalInput").ap()
    xf8 = nc.dram_tensor(
        "xf8", [B // G, P, G * NF8 * S], u8, kind="ExternalInput"
    ).ap()
    w = nc.dram_tensor("ws", [P, CB * S], bf16, kind="ExternalInput").ap()
    bias = nc.dram_tensor("bs", [1, S], bf16, kind="ExternalInput").ap()
    out = nc.dram_tensor("out", [B, S], f32, kind="ExternalOutput").ap()

    F8 = NF8 * S

    with tile.TileContext(nc) as tc:
        with (
            tc.tile_pool(name="const", bufs=1) as cpool,
            tc.tile_pool(name="xb", bufs=4) as bfpool,
            tc.tile_pool(name="x8", bufs=2) as f8pool,
            tc.tile_pool(name="up", bufs=4) as uppool,
            tc.tile_pool(name="ps", bufs=6, space="PSUM") as pspool,
            tc.tile_pool(name="op", bufs=1) as opool,
        ):
            w_sb = cpool.tile([P, CB * S], bf16)
            nc.sync.dma_start(w_sb[:], w[:])

            ones_f32 = cpool.tile([P, 1], f32)
            nc.vector.memset(ones_f32[:], 1.0)
            ones = cpool.tile([P, 1], bf16)
            nc.vector.tensor_copy(ones[:], ones_f32[:])

            bias_sb = cpool.tile([1, S], bf16)
            nc.scalar.dma_start(bias_sb[:], bias[:])

            out_sb = opool.tile([1, B * S], f32)

            relu_q = []

            def emit_relu(bq, psq):
                nc.scalar.activation(
                    out_sb[0:1, bq * S : (bq + 1) * S],
                    psq[:],
                    mybir.ActivationFunctionType.Relu,
                )
                if bq == EARLY - 1:
                    nc.scalar.dma_start(
                        out[0:EARLY].unsqueeze(0),
                        out_sb[:, 0 : EARLY * S].rearrange(
                            "p (b s) -> p b s", b=EARLY
                        ),
                    )

            W8 = NBF * S  # free offset of the fp8 blocks inside w

            # Software-pipelined emission: batch b's up-multiply and its
            # whole PE accumulation group are emitted one batch late, so
            # the DVE queue is [.., bf-mult(b), up-mult(b-1), ..] — when
            # bf-DMA(b) completion (+900 ns semaphore propagation) lags,
            # DVE always has the ready up-mult(b-1) to execute instead of
            # idling, and the PE group never waits on a just-issued mult.
            def finish(bq, t_bfq, t_upq):
                nc.vector.tensor_mul(
                    t_upq[:], t_upq[:], w_sb[:, W8 : W8 + F8]
                )
                ps = pspool.tile([1, S], f32)
                nc.tensor.matmul(
                    ps[:], ones[0:1, 0:1], bias_sb[:], start=True, stop=False
                )
                for j in range(NBF):
                    nc.tensor.matmul(
                        ps[:], ones[:], t_bfq[:, j * S : (j + 1) * S],
                        start=False, stop=False,
                    )
                for j in range(NF8):
                    nc.tensor.matmul(
                        ps[:], ones[:], t_upq[:, j * S : (j + 1) * S],
                        start=False, stop=(j == NF8 - 1),
                    )
                relu_q.append((bq, ps))
                if len(relu_q) > RELU_LAG:
                    emit_relu(*relu_q.pop(0))

            t_f8 = None
            prev = None
            for b in range(B):
                if b % G == 0:
                    t_f8 = f8pool.tile([P, G * F8], e3, tag="f8")
                    nc.sync.dma_start(
                        t_f8[:], xf8[b // G].bitcast(e3)
                    )
                g = (b % G) * F8

                t_bf = bfpool.tile([P, NBF * S], bf16)
                t_up = uppool.tile([P, F8], bf16)
                nc.sync.dma_start(t_bf[:], xbf[b])
                nc.scalar.activation(
                    t_up[:],
                    t_f8[:, g : g + F8],
                    mybir.ActivationFunctionType.Copy,
                )
                nc.vector.tensor_mul(t_bf[:], t_bf[:], w_sb[:, 0 : NBF * S])
                if prev is not None:
                    finish(*prev)
                prev = (b, t_bf, t_up)

            finish(*prev)
            for bq, psq in relu_q:
                emit_relu(bq, psq)

            nc.scalar.dma_start(
                out[EARLY:].unsqueeze(0),
                out_sb[:, EARLY * S :].rearrange(
                    "p (b s) -> p b s", b=B - EARLY
                ),
            )

    nc.compile()
    return nc


def _get_nc():
    if "nc" not in _nc_cache:
        _nc_cache["nc"] = _build()
    return _nc_cache["nc"]


def _e3m4_encode(v):
    """float32 -> e3m4 bits (uint8), round to nearest, clamp to +-15.5."""
    codes = np.arange(112, dtype=np.uint8)
    e = (codes >> 4) & 0x7
    m = codes & 0xF
    vals = np.where(e == 0, m * 2.0 ** (-6), (1 + m / 16.0) * 2.0 ** (e - 3.0))
    mids = (vals[1:] + vals[:-1]) / 2
    a = np.abs(v).astype(np.float32)
    code = np.searchsorted(mids, a).astype(np.uint8)
    return code | (np.signbit(v).astype(np.uint8) << 7)


def _shard_inputs(x, weights, bias):
    bf16 = ml_dtypes.bfloat16
    x = np.asarray(x, dtype=np.float32)
    weights = np.asarray(weights, dtype=np.float32)
    bias = np.asarray(bias, dtype=np.float32)
    nbf_c = NBF * P
    xb = x[:, :nbf_c, :].astype(bf16)
    x8 = _e3m4_encode(x[:, nbf_c:, :])
    wb = weights.astype(bf16)
    bb = bias.astype(bf16)
    in_maps = []
    for i in range(N_CORES):
        sl = slice(i * S, (i + 1) * S)
        # c = cb*P + p; pack [.., P, CB, S] so partition lines are contiguous
        xbi = xb[:, :, sl].reshape(B, NBF, P, S).transpose(0, 2, 1, 3)
        # fp8 grouped: [B/G, P, G, NF8, S]
        x8i = (
            x8[:, :, sl]
            .reshape(B // G, G, NF8, P, S)
            .transpose(0, 3, 1, 2, 4)
        )
        wi = wb[:, sl].reshape(CB, P, S).transpose(1, 0, 2)
        in_maps.append(
            {
                "xbf": np.ascontiguousarray(xbi).reshape(B, P, NBF * S),
                "xf8": np.ascontiguousarray(x8i).reshape(
                    B // G, P, G * NF8 * S
                ),
                "ws": np.ascontiguousarray(wi).reshape(P, CB * S),
                "bs": np.ascontiguousarray(bb[sl].reshape(1, S)),
            }
        )
    return in_maps


def _run(inputs, trace=False, trace_cores=None):
    from concourse import bass_utils

    nc = _get_nc()
    in_maps = _shard_inputs(inputs["x"], inputs["weights"], inputs["bias"])
    res = bass_utils.run_bass_kernel_spmd(
        nc,
        in_maps,
        core_ids=list(range(N_CORES)),
        trace=trace,
        trace_cores=trace_cores,
    )
    out = np.concatenate([r["out"] for r in res.results], axis=1)
    return out, res


def kernel(x, weights, bias):
    out, _ = _run({"x": x, "weights": weights, "bias": bias})
    return out


# revision 15
# speedup vs baseline: 1.1509x; 1.1509x over previous
"""Trainium2 Bass kernel for: out = relu(einsum('bcs,cs->bs', x, w) + bias).

Full shapes: x [32, 2048, 4096] f32, w [2048, 4096] f32, bias [4096] f32.
Sharding: the s-axis (4096) is split across 8 cores (512 each); each core
produces out[:, s_slice], gather = concat.

The kernel is HBM-bound (per-core DMA caps at ~400 GB/s regardless of
queue count), so precision is spent where it buys bandwidth: channel
blocks 0..9 ship as bf16, blocks 10..15 as fp8 e3m4 (1-3-4, bias 3,
exp=7 reserved -> clamp at 15.5). Measured end-to-end rel l2 error
~8.6e-3 against the 2e-2 budget. Per-core traffic: 54 MiB.

Engine split per batch (partitions = channel-within-block, free = cb*s):
  sync DMA  fp8 tiles grouped 4 batches per transfer (12 KiB lines),
            bf16 tile per batch (10 KiB lines)
  ACT       upconvert fp8 [128, 3072] -> bf16 staging (fp8 runs 1x on
            DVE, so the idle ACT engine eats the cast instead)
  DVE       xbf *= w[0:10]; up *= w[10:16]   (bf16 2x mode; the Pool
            engine measured 4.6 ns/elem on tensor ops plus heavy SBUF
            interference, so it gets no multiply work)
  PE        bf16 ones-matmul per c-block accumulating the 128-partition
            reduction into PSUM [1,512]; bf16 K=1 bias matmul opens the
            group
  ACT       relu PSUM -> out_sb row b (deferred 3 batches so its
            wait-on-PE cannot stall the upconvert chain)
  scalar DMA out rows 0..23 drained mid-stream, 24..31 at the end
"""

import numpy as np
import ml_dtypes

B, C, S_FULL = 32, 2048, 4096
N_CORES = 8
S = S_FULL // N_CORES          # 512 s-values per core
P = 128                        # SBUF partitions
CB = C // P                    # 16 channel blocks
NBF = 10                       # blocks 0..9 bf16
NF8 = CB - NBF                 # blocks 10..15 fp8 e3m4
G = 4                          # batches per fp8 DMA group
RELU_LAG = 3
EARLY = 24                     # rows drained mid-stream

_nc_cache = {}


def _build():
    import concourse.bacc as bacc
    import concourse.mybir as mybir
    import concourse.tile as tile

    f32 = mybir.dt.float32
    bf16 = mybir.dt.bfloat16
    e3 = mybir.dt.float8e3
    u8 = mybir.dt.uint8
    nc = bacc.Bacc(
        "TRN2",
        target_bir_lowering=False,
        debug=False,
        enable_asserts=False,
        num_devices=N_CORES,
    )

    # Host pre-packs everything into SBUF layout. fp8 is stored grouped:
    # [B/G, P, G*NF8*S] so one DMA covers G batches with 12 KiB lines.
    xbf = nc.dram_tensor("xbf", [B, P, NBF * S], bf16, kind="ExternalInput").ap()
    xf8 = nc.dram_tensor(
        "xf8", [B // G, P, G * NF8 * S], u8, kind="ExternalInput"
    ).ap()
    w = nc.dram_tensor("ws", [P, CB * S], bf16, kind="ExternalInput").ap()
    bias = nc.dram_tensor("bs", [1, S], bf16, kind="ExternalInput").ap()
    out = nc.dram_tensor("out", [B, S], f32, kind="ExternalOutput").ap()

    F8 = NF8 * S

    with tile.TileContext(nc) as tc:
        with (
            tc.tile_pool(name="const", bufs=1) as cpool,
            tc.tile_pool(name="xb", bufs=6) as bfpool,
            tc.tile_pool(name="x8", bufs=2) as f8pool,
            tc.tile_pool(name="up", bufs=5) as uppool,
            tc.tile_pool(name="ps", bufs=6, space="PSUM") as pspool,
            tc.tile_pool(name="op", bufs=1) as opool,
        ):
            w_sb = cpool.tile([P, CB * S], bf16)
            nc.sync.dma_start(w_sb[:], w[:])

            ones_f32 = cpool.tile([P, 1], f32)
            nc.vector.memset(ones_f32[:], 1.0)
            ones = cpool.tile([P, 1], bf16)
            nc.vector.tensor_copy(ones[:], ones_f32[:])

            bias_sb = cpool.tile([1, S], bf16)
            nc.scalar.dma_start(bias_sb[:], bias[:])

            out_sb = opool.tile([1, B * S], f32)

            relu_q = []

            def emit_relu(bq, psq):
                nc.scalar.activation(
                    out_sb[0:1, bq * S : (bq + 1) * S],
                    psq[:],
                    mybir.ActivationFunctionType.Relu,
                )
                if bq == EARLY - 1:
                    nc.scalar.dma_start(
                        out[0:EARLY].unsqueeze(0),
                        out_sb[:, 0 : EARLY * S].rearrange(
                            "p (b s) -> p b s", b=EARLY
                        ),
                    )

            W8 = NBF * S  # free offset of the fp8 blocks inside w

            # Software-pipelined emission: batch b's up-multiply and its
            # whole PE accumulation group are emitted one batch late, so
            # the DVE queue is [.., bf-mult(b), up-mult(b-1), ..] — when
            # bf-DMA(b) completion (+900 ns semaphore propagation) lags,
            # DVE always has the ready up-mult(b-1) to execute instead of
            # idling, and the PE group never waits on a just-issued mult.
            def finish(bq, t_bfq, t_upq):
                nc.vector.tensor_mul(
                    t_upq[:], t_upq[:], w_sb[:, W8 : W8 + F8]
                )
                ps = pspool.tile([1, S], f32)
                nc.tensor.matmul(
                    ps[:], ones[0:1, 0:1], bias_sb[:], start=True, stop=False
                )
                for j in range(NBF):
                    nc.tensor.matmul(
                        ps[:], ones[:], t_bfq[:, j * S : (j + 1) * S],
                        start=False, stop=False,
                    )
                for j in range(NF8):
                    nc.tensor.matmul(
                        ps[:], ones[:], t_upq[:, j * S : (j + 1) * S],
                        start=False, stop=(j == NF8 - 1),
                    )
                relu_q.append((bq, ps))
                if len(relu_q) > RELU_LAG:
                    emit_relu(*relu_q.pop(0))

            t_f8 = None
            prev = None
            for b in range(B):
                if b % G == 0:
                    t_f8 = f8pool.tile([P, G * F8], e3, tag="f8")
                    nc.sync.dma_start(
                        t_f8[:], xf8[b // G].bitcast(e3)
                    )
                g = (b % G) * F8

                t_bf = bfpool.tile([P, NBF * S], bf16)
                t_up = uppool.tile([P, F8], bf16)
                nc.sync.dma_start(t_bf[:], xbf[b])
                nc.scalar.activation(
                    t_up[:],
                    t_f8[:, g : g + F8],
                    mybir.ActivationFunctionType.Copy,
                )
                nc.vector.tensor_mul(t_bf[:], t_bf[:], w_sb[:, 0 : NBF * S])
                if prev is not None:
                    finish(*prev)
                prev = (b, t_bf, t_up)

            finish(*prev)
            for bq, psq in relu_q:
                emit_relu(bq, psq)

            nc.scalar.dma_start(
                out[EARLY:].unsqueeze(0),
                out_sb[:, EARLY * S :].rearrange(
                    "p (b s) -> p b s", b=B - EARLY
                ),
            )

    nc.compile()
    return nc


def _get_nc():
    if "nc" not in _nc_cache:
        _nc_cache["nc"] = _build()
    return _nc_cache["nc"]


def _e3m4_encode(v):
    """float32 -> e3m4 bits (uint8), round to nearest, clamp to +-15.5."""
    codes = np.arange(112, dtype=np.uint8)
    e = (codes >> 4) & 0x7
    m = codes & 0xF
    vals = np.where(e == 0, m * 2.0 ** (-6), (1 + m / 16.0) * 2.0 ** (e - 3.0))
    mids = (vals[1:] + vals[:-1]) / 2
    a = np.abs(v).astype(np.float32)
    code = np.searchsorted(mids, a).astype(np.uint8)
    return code | (np.signbit(v).astype(np.uint8) << 7)


def _shard_inputs(x, weights, bias):
    bf16 = ml_dtypes.bfloat16
    x = np.asarray(x, dtype=np.float32)
    weights = np.asarray(weights, dtype=np.float32)
    bias = np.asarray(bias, dtype=np.float32)
    nbf_c = NBF * P
    xb = x[:, :nbf_c, :].astype(bf16)
    x8 = _e3m4_encode(x[:, nbf_c:, :])
    wb = weights.astype(bf16)
    bb = bias.astype(bf16)
    in_maps = []
    for i in range(N_CORES):
        sl = slice(i * S, (i + 1) * S)
        # c = cb*P + p; pack [.., P, CB, S] so partition lines are contiguous
        xbi = xb[:, :, sl].reshape(B, NBF, P, S).transpose(0, 2, 1, 3)
        # fp8 grouped: [B/G, P, G, NF8, S]
        x8i = (
            x8[:, :, sl]
            .reshape(B // G, G, NF8, P, S)
            .transpose(0, 3, 1, 2, 4)
        )
        wi = wb[:, sl].reshape(CB, P, S).transpose(1, 0, 2)
        in_maps.append(
            {
                "xbf": np.ascontiguousarray(xbi).reshape(B, P, NBF * S),
                "xf8": np.ascontiguousarray(x8i).reshape(
                    B // G, P, G * NF8 * S
                ),
                "ws": np.ascontiguousarray(wi).reshape(P, CB * S),
                "bs": np.ascontiguousarray(bb[sl].reshape(1, S)),
            }
        )
    return in_maps


def _run(inputs, trace=False, trace_cores=None):
    from concourse import bass_utils

    nc = _get_nc()
    in_maps = _shard_inputs(inputs["x"], inputs["weights"], inputs["bias"])
    res = bass_utils.run_bass_kernel_spmd(
        nc,
        in_maps,
        core_ids=list(range(N_CORES)),
        trace=trace,
        trace_cores=trace_cores,
    )
    out = np.concatenate([r["out"] for r in res.results], axis=1)
    return out, res


def kernel(x, weights, bias):
    out, _ = _run({"x": x, "weights": weights, "bias": bias})
    return out


# revision 16
# speedup vs baseline: 1.2385x; 1.0760x over previous
"""Trainium2 Bass kernel for: out = relu(einsum('bcs,cs->bs', x, w) + bias).

Full shapes: x [32, 2048, 4096] f32, w [2048, 4096] f32, bias [4096] f32.
Sharding: the s-axis (4096) is split across 8 cores (512 each); each core
produces out[:, s_slice], gather = concat.

The kernel is HBM-bound (per-core DMA caps at ~400 GB/s regardless of
queue count), so precision is spent where it buys bandwidth: channel
blocks 0..9 ship as bf16, blocks 10..15 as fp8 e3m4 (1-3-4, bias 3,
exp=7 reserved -> clamp at 15.5). Measured end-to-end rel l2 error
~8.6e-3 against the 2e-2 budget. Per-core traffic: 54 MiB.

Engine split per batch (partitions = channel-within-block, free = cb*s):
  sync DMA  fp8 tiles grouped 4 batches per transfer (12 KiB lines),
            bf16 tile per batch (10 KiB lines)
  ACT       upconvert fp8 [128, 3072] -> bf16 staging (fp8 runs 1x on
            DVE, so the idle ACT engine eats the cast instead)
  DVE       xbf *= w[0:10]; up *= w[10:16]   (bf16 2x mode; the Pool
            engine measured 4.6 ns/elem on tensor ops plus heavy SBUF
            interference, so it gets no multiply work)
  PE        bf16 ones-matmul per c-block accumulating the 128-partition
            reduction into PSUM [1,512]; bf16 K=1 bias matmul opens the
            group
  ACT       relu PSUM -> out_sb row b (deferred 3 batches so its
            wait-on-PE cannot stall the upconvert chain)
  scalar DMA out rows 0..23 drained mid-stream, 24..31 at the end
"""

import numpy as np
import ml_dtypes

B, C, S_FULL = 32, 2048, 4096
N_CORES = 8
S = S_FULL // N_CORES          # 512 s-values per core
P = 128                        # SBUF partitions
CB = C // P                    # 16 channel blocks
NBF = 10                       # blocks 0..9 bf16
NF8 = CB - NBF                 # blocks 10..15 fp8 e3m4
G = 4                          # batches per fp8 DMA group
RELU_LAG = 3
EARLY = 24                     # rows drained mid-stream

_nc_cache = {}


def _build():
    import concourse.bacc as bacc
    import concourse.mybir as mybir
    import concourse.tile as tile

    f32 = mybir.dt.float32
    bf16 = mybir.dt.bfloat16
    e3 = mybir.dt.float8e3
    u8 = mybir.dt.uint8
    nc = bacc.Bacc(
        "TRN2",
        target_bir_lowering=False,
        debug=False,
        enable_asserts=False,
        num_devices=N_CORES,
    )

    # Host pre-packs everything into SBUF layout. fp8 is stored grouped:
    # [B/G, P, G*NF8*S] so one DMA covers G batches with 12 KiB lines.
    xbf = nc.dram_tensor(
        "xbf", [B // 2, P, 2 * NBF * S], bf16, kind="ExternalInput"
    ).ap()
    xf8 = nc.dram_tensor(
        "xf8", [B // G, P, G * NF8 * S], u8, kind="ExternalInput"
    ).ap()
    w = nc.dram_tensor("ws", [P, CB * S], bf16, kind="ExternalInput").ap()
    bias = nc.dram_tensor("bs", [1, S], bf16, kind="ExternalInput").ap()
    out = nc.dram_tensor("out", [B, S], f32, kind="ExternalOutput").ap()

    F8 = NF8 * S

    with tile.TileContext(nc) as tc:
        with (
            tc.tile_pool(name="const", bufs=1) as cpool,
            tc.tile_pool(name="xb", bufs=3) as bfpool,
            tc.tile_pool(name="x8", bufs=2) as f8pool,
            tc.tile_pool(name="up", bufs=5) as uppool,
            tc.tile_pool(name="ps", bufs=6, space="PSUM") as pspool,
            tc.tile_pool(name="op", bufs=1) as opool,
        ):
            w_sb = cpool.tile([P, CB * S], bf16)
            nc.sync.dma_start(w_sb[:], w[:])

            ones_f32 = cpool.tile([P, 1], f32)
            nc.vector.memset(ones_f32[:], 1.0)
            ones = cpool.tile([P, 1], bf16)
            nc.vector.tensor_copy(ones[:], ones_f32[:])

            bias_sb = cpool.tile([1, S], bf16)
            nc.scalar.dma_start(bias_sb[:], bias[:])

            out_sb = opool.tile([1, B * S], f32)

            relu_q = []

            def emit_relu(bq, psq):
                nc.scalar.activation(
                    out_sb[0:1, bq * S : (bq + 1) * S],
                    psq[:],
                    mybir.ActivationFunctionType.Relu,
                )
                if bq == EARLY - 1:
                    nc.scalar.dma_start(
                        out[0:EARLY].unsqueeze(0),
                        out_sb[:, 0 : EARLY * S].rearrange(
                            "p (b s) -> p b s", b=EARLY
                        ),
                    )

            W8 = NBF * S  # free offset of the fp8 blocks inside w

            # Software-pipelined emission: batch b's up-multiply and its
            # whole PE accumulation group are emitted one batch late, so
            # the DVE queue is [.., bf-mult(b), up-mult(b-1), ..] — when
            # bf-DMA(b) completion (+900 ns semaphore propagation) lags,
            # DVE always has the ready up-mult(b-1) to execute instead of
            # idling, and the PE group never waits on a just-issued mult.
            def finish(bq, t_bfq, t_upq):
                nc.vector.tensor_mul(
                    t_upq[:], t_upq[:], w_sb[:, W8 : W8 + F8]
                )
                ps = pspool.tile([1, S], f32)
                nc.tensor.matmul(
                    ps[:], ones[0:1, 0:1], bias_sb[:], start=True, stop=False
                )
                for j in range(NBF):
                    nc.tensor.matmul(
                        ps[:], ones[:], t_bfq[:, j * S : (j + 1) * S],
                        start=False, stop=False,
                    )
                for j in range(NF8):
                    nc.tensor.matmul(
                        ps[:], ones[:], t_upq[:, j * S : (j + 1) * S],
                        start=False, stop=(j == NF8 - 1),
                    )
                relu_q.append((bq, ps))
                if len(relu_q) > RELU_LAG:
                    emit_relu(*relu_q.pop(0))

            BFT = NBF * S
            t_f8 = None
            t_bf2 = None
            prev = None
            for b in range(B):
                if b % G == 0:
                    t_f8 = f8pool.tile([P, G * F8], e3, tag="f8")
                    nc.sync.dma_start(
                        t_f8[:], xf8[b // G].bitcast(e3)
                    )
                g = (b % G) * F8
                if b % 2 == 0:
                    # two batches per transfer: 20 KiB partition lines
                    # measure ~60 GB/s faster than 10 KiB ones
                    t_bf2 = bfpool.tile([P, 2 * BFT], bf16, tag="bf2")
                    nc.sync.dma_start(t_bf2[:], xbf[b // 2])
                t_bf = t_bf2[:, (b % 2) * BFT : (b % 2 + 1) * BFT]

                t_up = uppool.tile([P, F8], bf16)
                nc.scalar.activation(
                    t_up[:],
                    t_f8[:, g : g + F8],
                    mybir.ActivationFunctionType.Copy,
                )
                nc.vector.tensor_mul(t_bf, t_bf, w_sb[:, 0 : NBF * S])
                if prev is not None:
                    finish(*prev)
                prev = (b, t_bf, t_up)

            finish(*prev)
            for bq, psq in relu_q:
                emit_relu(bq, psq)

            nc.scalar.dma_start(
                out[EARLY:].unsqueeze(0),
                out_sb[:, EARLY * S :].rearrange(
                    "p (b s) -> p b s", b=B - EARLY
                ),
            )

    nc.compile()
    return nc


def _get_nc():
    if "nc" not in _nc_cache:
        _nc_cache["nc"] = _build()
    return _nc_cache["nc"]


def _e3m4_encode(v):
    """float32 -> e3m4 bits (uint8), round to nearest, clamp to +-15.5."""
    codes = np.arange(112, dtype=np.uint8)
    e = (codes >> 4) & 0x7
    m = codes & 0xF
    vals = np.where(e == 0, m * 2.0 ** (-6), (1 + m / 16.0) * 2.0 ** (e - 3.0))
    mids = (vals[1:] + vals[:-1]) / 2
    a = np.abs(v).astype(np.float32)
    code = np.searchsorted(mids, a).astype(np.uint8)
    return code | (np.signbit(v).astype(np.uint8) << 7)


def _shard_inputs(x, weights, bias):
    bf16 = ml_dtypes.bfloat16
    x = np.asarray(x, dtype=np.float32)
    weights = np.asarray(weights, dtype=np.float32)
    bias = np.asarray(bias, dtype=np.float32)
    nbf_c = NBF * P
    xb = x[:, :nbf_c, :].astype(bf16)
    x8 = _e3m4_encode(x[:, nbf_c:, :])
    wb = weights.astype(bf16)
    bb = bias.astype(bf16)
    in_maps = []
    for i in range(N_CORES):
        sl = slice(i * S, (i + 1) * S)
        # c = cb*P + p; pack [.., P, CB, S] so partition lines are contiguous
        xbi = (
            xb[:, :, sl]
            .reshape(B // 2, 2, NBF, P, S)
            .transpose(0, 3, 1, 2, 4)
        )
        # fp8 grouped: [B/G, P, G, NF8, S]
        x8i = (
            x8[:, :, sl]
            .reshape(B // G, G, NF8, P, S)
            .transpose(0, 3, 1, 2, 4)
        )
        wi = wb[:, sl].reshape(CB, P, S).transpose(1, 0, 2)
        in_maps.append(
            {
                "xbf": np.ascontiguousarray(xbi).reshape(
                    B // 2, P, 2 * NBF * S
                ),
                "xf8": np.ascontiguousarray(x8i).reshape(
                    B // G, P, G * NF8 * S
                ),
                "ws": np.ascontiguousarray(wi).reshape(P, CB * S),
                "bs": np.ascontiguousarray(bb[sl].reshape(1, S)),
            }
        )
    return in_maps


def _run(inputs, trace=False, trace_cores=None):
    from concourse import bass_utils

    nc = _get_nc()
    in_maps = _shard_inputs(inputs["x"], inputs["weights"], inputs["bias"])
    res = bass_utils.run_bass_kernel_spmd(
        nc,
        in_maps,
        core_ids=list(range(N_CORES)),
        trace=trace,
        trace_cores=trace_cores,
    )
    out = np.concatenate([r["out"] for r in res.results], axis=1)
    return out, res


def kernel(x, weights, bias):
    out, _ = _run({"x": x, "weights": weights, "bias": bias})
    return out
